# revision 24
# baseline (speedup 1.0000x reference)
"""HardMemory retrieval-KNN kernel for 8 Trainium2 NeuronCores.

Data-parallel: 32 batches sharded 4-per-core; memory bank [1024,512]
replicated. Per batch b (x_b = [C=512, N=4096]), processed in eight
512-pixel units, software-pipelined two units deep so no engine waits
on the cross-engine compare chain:

  round k emits:  A(k)  = DMA + squares + sumsq/sim fp8 DoubleRow
                          matmuls + psum->sbuf bf16 copies + DVE max
                          tree + gpsimd partition max
                  C1(k-1) = threshold fold + gpsimd broadcast
                  B(k-2)  = gather matmul (fp8 DR) + out copies + DMA
                  C2(k-1) = onehot compare (bf16 exact -> fp8)

  simT[m,n]  = <x_n, mem_m/||mem_m||>    fp8 DR matmul, f32 psum
  thr[n]     = 0.8*sqrt(sum_c x^2)       ones-stationary fp8 DR matmul
  cm[n]      = colmax_m bf16(simT)       DVE bf16 2x + gpsimd reduce
  mx'[n]     = cm - BIG*(cm <= thr)      mask folded into compare value
  oh[m,n]    = (bf16(simT) == bcast(mx'))
  out[:,n]   = memory^T @ oh             fp8 DR matmul -> bf16 out

x arrives as fp8e4m3 (host cast): halves input DMA and enables the
DoubleRow similarity matmul.  Cosine margins are huge vs fp8 noise
(|sim| <= ~6 vs thr ~18 for randn inputs), and the bf16 compare domain
is exact by construction (max of bf16 values == some bf16 value).
"""

import sys

for _p in ("/opt/trn_rl_repo",):
    if _p not in sys.path:
        sys.path.insert(0, _p)

from contextlib import ExitStack

import ml_dtypes
import numpy as np

import concourse.bass as bass
import concourse.tile as tile
from concourse import bacc, bass_isa, mybir
from concourse.bass_utils import run_bass_kernel_spmd

F32 = mybir.dt.float32
BF16 = mybir.dt.bfloat16
FP8 = mybir.dt.float8e4
AF = mybir.ActivationFunctionType
ALU = mybir.AluOpType
DR = mybir.MatmulPerfMode.DoubleRow

B_FULL, C, H, W = 32, 512, 64, 64
N_PIX = H * W
M = 1024
N_CORES = 8
B_LOC = B_FULL // N_CORES
THRESH2 = 0.8 * 0.8
BIG = 1.0e30

MC = M // 128            # 8 memory chunks
MJ = MC // 2             # 4 DoubleRow memory pairs
CJ = C // 256            # 2 DoubleRow contraction pairs


def build_kernel(b_loc=B_LOC, n_pix=N_PIX):
    ns_count = n_pix // 512

    nc = bacc.Bacc("TRN2", target_bir_lowering=False, debug=False,
                   num_devices=N_CORES)
    xs = nc.dram_tensor("xs", [b_loc, C, n_pix], FP8, kind="ExternalInput")
    mem = nc.dram_tensor("memory", [M, C], F32, kind="ExternalInput")
    ident_b = nc.dram_tensor("identity", [128, 128], BF16, kind="ExternalInput")
    out = nc.dram_tensor("out", [b_loc, C, n_pix], BF16,
                         kind="ExternalOutput")

    with tile.TileContext(nc) as tc, ExitStack() as ctx:
        const = ctx.enter_context(tc.tile_pool(name="const", bufs=1))
        mstage = ctx.enter_context(tc.tile_pool(name="mstage", bufs=2))
        mtmp = ctx.enter_context(tc.tile_pool(name="mtmp", bufs=2))
        xio = ctx.enter_context(tc.tile_pool(name="xio", bufs=6))
        simb = ctx.enter_context(tc.tile_pool(name="simb", bufs=4))
        ohb = ctx.enter_context(tc.tile_pool(name="ohb", bufs=4))
        stats = ctx.enter_context(tc.tile_pool(name="stats", bufs=6))
        # psum (8 banks): sim 2x[128,2,512]f32 (4) + b1 4x[128,512]f32 (4);
        # preproc transposes ride the b1 ring.
        psum = ctx.enter_context(
            tc.tile_pool(name="psum", bufs=1, space=bass.MemorySpace.PSUM))

        idb = const.tile([128, 128], BF16, tag="idb")
        nc.sync.dma_start(idb[:], ident_b[:])
        ones2 = const.tile([128, 2, 128], FP8, tag="ones2")
        nc.gpsimd.memset(ones2[:], 1.0)

        # ---- memory preprocessing ----
        # Dual-fp8 ldweights needs each [2, 128] stationary block contiguous.
        # memS2[mj][p, ci, i, c] = mem[(2mj+i)*128+p, ci*128+c]   (mm2 lhsT)
        # memT2[cj][p, mt, i, m] = mem_norm[mt*128+m, (2cj+i)*128+p] (mm1 lhsT)
        memS2 = [const.tile([128, C // 128, 2, 128], FP8, tag=f"memS2_{mj}",
                            name=f"memS2_{mj}") for mj in range(MJ)]
        memT2 = [const.tile([128, MC, 2, 128], FP8, tag=f"memT2_{cj}",
                            name=f"memT2_{cj}") for cj in range(CJ)]
        for mi in range(MC):
            mld = mstage.tile([128, C], F32, tag="mld")
            nc.sync.dma_start(mld[:], mem[mi * 128:(mi + 1) * 128, :])
            msq = mtmp.tile([128, C], F32, tag="msq")
            mssq = stats.tile([128, 1], F32, tag="mssq")
            nc.scalar.activation(msq[:], mld[:], AF.Square, accum_out=mssq[:])
            mnorm = stats.tile([128, 1], F32, tag="mnorm")
            nc.scalar.activation(mnorm[:], mssq[:], AF.Sqrt)
            rinv = stats.tile([128, 1], F32, tag="rinv")
            nc.vector.reciprocal(rinv[:], mnorm[:])
            nc.scalar.activation(memS2[mi // 2][:, :, mi % 2, :], mld[:],
                                 AF.Copy)
            mn = mtmp.tile([128, C], BF16, tag="mn")
            nc.vector.tensor_scalar_mul(mn[:], mld[:], rinv[:])
            for ci in range(C // 128):
                ptr = psum.tile([128, 128], BF16, tag="b1", bufs=4,
                                name="ptr")
                nc.tensor.transpose(ptr[:], mn[:, ci * 128:(ci + 1) * 128],
                                    idb[:])
                nc.scalar.activation(
                    memT2[ci // 2][:, mi, ci % 2, :], ptr[:], AF.Copy)

        def phase_dma(b, ns):
            """Prefetch one unit's input (issued a round early)."""
            n0 = ns * 512
            x4 = xio.tile([128, 4, 512], FP8, tag="x4", name="x4")
            for ch in range(4):
                nc.sync.dma_start(x4[:, ch, :],
                                  xs[b, ch * 128:(ch + 1) * 128, n0:n0 + 512])
            return {"b": b, "ns": ns, "x4": x4}

        def phase_a0(st):
            """Squares (scalar, at round start so sumsq can issue early)."""
            xq4 = xio.tile([128, 4, 512], FP8, tag="xq4", name="xq4")
            nc.scalar.activation(xq4[:], st["x4"][:], AF.Square)
            st["xq4"] = xq4

        def phase_a1(st):
            """Sumsq + sim matmuls + copies + column max."""
            x4, xq4 = st["x4"], st["xq4"]
            pq = psum.tile([128, 512], F32, tag="b1", bufs=4, name="pq")
            for cj in range(CJ):
                nc.tensor.matmul(pq[:], ones2[:],
                                 xq4[:, 2 * cj:2 * cj + 2, :],
                                 start=(cj == 0), stop=(cj == CJ - 1),
                                 perf_mode=DR)
            sTb = simb.tile([128, MJ, 2, 512], BF16, tag="sTb", name="sTb")
            cmp4 = stats.tile([128, MJ, 512], BF16, tag="cmp4")
            for mj in range(MJ):
                ps = psum.tile([128, 2, 512], F32, tag="sim", bufs=2,
                               name="ps")
                for i in range(2):
                    mt = 2 * mj + i
                    for cj in range(CJ):
                        nc.tensor.matmul(
                            ps[:, i, :], memT2[cj][:, mt, :, :],
                            x4[:, 2 * cj:2 * cj + 2, :],
                            start=(cj == 0), stop=(cj == CJ - 1),
                            perf_mode=DR)
                nc.scalar.activation(sTb[:, mj, :, :], ps[:], AF.Copy)
                nc.vector.tensor_tensor(cmp4[:, mj, :], sTb[:, mj, 0, :],
                                        sTb[:, mj, 1, :], ALU.max)
            cmx2 = stats.tile([128, 2, 512], BF16, tag="cmx2")
            nc.vector.tensor_tensor(cmx2[:], cmp4[:, 0:2, :], cmp4[:, 2:4, :],
                                    ALU.max)
            cm = stats.tile([128, 512], BF16, tag="cm")
            nc.vector.tensor_tensor(cm[:], cmx2[:, 0, :], cmx2[:, 1, :],
                                    ALU.max)
            cmB = stats.tile([128, 512], BF16, tag="cmB", bufs=2)
            nc.gpsimd.partition_all_reduce(cmB[:], cm[:], 128,
                                           bass_isa.ReduceOp.max)
            thr = stats.tile([1, 512], F32, tag="thr")
            nc.scalar.activation(thr[:], pq[0:1, :], AF.Sqrt, scale=THRESH2)
            st["sTb"], st["cmB"], st["thr"] = sTb, cmB, thr

        def phase_c1(st):
            """Fold mask into compare value, broadcast across partitions."""
            cmB, thr = st["cmB"], st["thr"]
            msk = stats.tile([1, 512], F32, tag="msk")
            nc.vector.tensor_tensor(msk[:], cmB[0:1, :], thr[:], ALU.is_le)
            mxrow = stats.tile([1, 512], BF16, tag="mxrow")
            nc.vector.scalar_tensor_tensor(mxrow[:], msk[:], -BIG,
                                           cmB[0:1, :], ALU.mult, ALU.add)
            mxB = stats.tile([128, 512], BF16, tag="mxB", bufs=2)
            nc.gpsimd.partition_broadcast(mxB[:], mxrow[:], 128)
            st["mxB"] = mxB

        def phase_c2(st):
            """Onehot: exact bf16 compare -> fp8 (one mega op)."""
            oh = ohb.tile([128, MJ, 2, 512], FP8, tag="oh", name="oh")
            mxv = st["mxB"][:].unsqueeze(1).unsqueeze(1).broadcast_to(
                [128, MJ, 2, 512])
            nc.vector.tensor_tensor(oh[:], st["sTb"][:], mxv, ALU.is_equal)
            st["oh"] = oh

        def phase_b(st):
            """out[c, n] = sum_m mem[m, c] * onehot[m, n] -> DMA out."""
            b, ns, oh = st["b"], st["ns"], st["oh"]
            n0 = ns * 512
            for ci in range(C // 128):
                pB = psum.tile([128, 512], F32, tag="b1", bufs=4, name="pB")
                for mj in range(MJ):
                    nc.tensor.matmul(
                        pB[:], memS2[mj][:, ci, :, :], oh[:, mj, :, :],
                        start=(mj == 0), stop=(mj == MJ - 1), perf_mode=DR)
                ob = ohb.tile([128, 512], BF16, tag="ob", bufs=4, name="ob")
                if ci < 2:
                    nc.scalar.activation(ob[:], pB[:], AF.Copy)
                else:
                    nc.vector.tensor_copy(ob[:], pB[:])
                nc.sync.dma_start(
                    out[b, ci * 128:(ci + 1) * 128, n0:n0 + 512], ob[:])

        # ---- main loop, software-pipelined two units deep ----
        # Round k: dma(k+1), xsq(k), fold+bcast(k-1), gather(k-2),
        #          onehot(k-1), sumsq/sim/max(k).  Each engine's in-order
        #          queue then always sees ready work first.
        units = [(b, ns) for b in range(b_loc) for ns in range(ns_count)]
        states = [None] * len(units)
        states[0] = phase_dma(*units[0])
        for k in range(len(units)):
            if k + 1 < len(units):
                states[k + 1] = phase_dma(*units[k + 1])
            phase_a0(states[k])
            if k >= 1:
                phase_c1(states[k - 1])
                phase_c2(states[k - 1])
            phase_a1(states[k])
            if k >= 2:
                phase_b(states[k - 2])
                states[k - 2] = None
        last = len(units) - 1
        phase_c1(states[last])
        if last >= 1:
            phase_b(states[last - 1])
        phase_c2(states[last])
        phase_b(states[last])

    nc.compile()
    return nc


_NC_CACHE = {}


def _get_nc(b_loc=B_LOC, n_pix=N_PIX):
    key = (b_loc, n_pix)
    if key not in _NC_CACHE:
        _NC_CACHE[key] = build_kernel(*key)
    return _NC_CACHE[key]


def run_on_hw(x_flat, memory, b_loc=B_LOC, n_pix=N_PIX, trace=False,
              **spmd_kwargs):
    """x_flat: [N_CORES*b_loc, C, n_pix] f32. Returns (out_full, results)."""
    nc = _get_nc(b_loc, n_pix)
    ident_b = np.eye(128, dtype=ml_dtypes.bfloat16)
    x_f8 = x_flat.astype(ml_dtypes.float8_e4m3)
    in_maps = [
        {
            "xs": np.ascontiguousarray(x_f8[c * b_loc:(c + 1) * b_loc]),
            "memory": memory,
            "identity": ident_b,
        }
        for c in range(N_CORES)
    ]
    res = run_bass_kernel_spmd(nc, in_maps, list(range(N_CORES)),
                               trace=trace, **spmd_kwargs)
    outs = [np.asarray(res.results[c]["out"]).astype(np.float32)
            for c in range(N_CORES)]
    return np.concatenate(outs, axis=0), res


def kernel(x, memory):
    x = np.asarray(x, dtype=np.float32)
    memory = np.asarray(memory, dtype=np.float32)
    B, C_, H_, W_ = x.shape
    x_flat = np.ascontiguousarray(x.reshape(B, C_, H_ * W_))
    out_flat, _ = run_on_hw(x_flat, memory)
    return out_flat.reshape(B, C_, H_, W_)


# revision 26
# speedup vs baseline: 1.0655x; 1.0655x over previous
"""HardMemory retrieval-KNN kernel for 8 Trainium2 NeuronCores.

Data-parallel: 32 batches sharded 4-per-core; memory bank [1024,512]
replicated. Per batch b (x_b = [C=512, N=4096]), processed in eight
512-pixel units, software-pipelined two units deep so no engine waits
on the cross-engine compare chain:

  round k emits:  A(k)  = DMA + squares + sumsq/sim fp8 DoubleRow
                          matmuls + psum->sbuf bf16 copies + DVE max
                          tree + gpsimd partition max
                  C1(k-1) = threshold fold + gpsimd broadcast
                  B(k-2)  = gather matmul (fp8 DR) + out copies + DMA
                  C2(k-1) = onehot compare (bf16 exact -> fp8)

  simT[m,n]  = <x_n, mem_m/||mem_m||>    fp8 DR matmul, f32 psum
  thr[n]     = 0.8*sqrt(sum_c x^2)       ones-stationary fp8 DR matmul
  cm[n]      = colmax_m bf16(simT)       DVE bf16 2x + gpsimd reduce
  mx'[n]     = cm - BIG*(cm <= thr)      mask folded into compare value
  oh[m,n]    = (bf16(simT) == bcast(mx'))
  out[:,n]   = memory^T @ oh             fp8 DR matmul -> bf16 out

x arrives as fp8e4m3 (host cast): halves input DMA and enables the
DoubleRow similarity matmul.  Cosine margins are huge vs fp8 noise
(|sim| <= ~6 vs thr ~18 for randn inputs), and the bf16 compare domain
is exact by construction (max of bf16 values == some bf16 value).
"""

import sys

for _p in ("/opt/trn_rl_repo",):
    if _p not in sys.path:
        sys.path.insert(0, _p)

from contextlib import ExitStack

import ml_dtypes
import numpy as np

import concourse.bass as bass
import concourse.tile as tile
from concourse import bacc, bass_isa, mybir
from concourse.bass_utils import run_bass_kernel_spmd

F32 = mybir.dt.float32
BF16 = mybir.dt.bfloat16
FP8 = mybir.dt.float8e4
AF = mybir.ActivationFunctionType
ALU = mybir.AluOpType
DR = mybir.MatmulPerfMode.DoubleRow

B_FULL, C, H, W = 32, 512, 64, 64
N_PIX = H * W
M = 1024
N_CORES = 8
B_LOC = B_FULL // N_CORES
THRESH2 = 0.8 * 0.8
BIG = 1.0e30

MC = M // 128            # 8 memory chunks
MJ = MC // 2             # 4 DoubleRow memory pairs
CJ = C // 256            # 2 DoubleRow contraction pairs


def build_kernel(b_loc=B_LOC, n_pix=N_PIX):
    ns_count = n_pix // 512

    nc = bacc.Bacc("TRN2", target_bir_lowering=False, debug=False,
                   num_devices=N_CORES)
    xs = nc.dram_tensor("xs", [b_loc, C, n_pix], FP8, kind="ExternalInput")
    mem = nc.dram_tensor("memory", [M, C], F32, kind="ExternalInput")
    ident_b = nc.dram_tensor("identity", [128, 128], BF16, kind="ExternalInput")
    out = nc.dram_tensor("out", [b_loc, C, n_pix], BF16,
                         kind="ExternalOutput")

    with tile.TileContext(nc) as tc, ExitStack() as ctx:
        const = ctx.enter_context(tc.tile_pool(name="const", bufs=1))
        mstage = ctx.enter_context(tc.tile_pool(name="mstage", bufs=2))
        mtmp = ctx.enter_context(tc.tile_pool(name="mtmp", bufs=2))
        xio = ctx.enter_context(tc.tile_pool(name="xio", bufs=6))
        simb = ctx.enter_context(tc.tile_pool(name="simb", bufs=4))
        ohb = ctx.enter_context(tc.tile_pool(name="ohb", bufs=4))
        stats = ctx.enter_context(tc.tile_pool(name="stats", bufs=6))
        # psum (8 banks): sim 2x[128,2,512]f32 (4) + b1 4x[128,512]f32 (4);
        # preproc transposes ride the b1 ring.
        psum = ctx.enter_context(
            tc.tile_pool(name="psum", bufs=1, space=bass.MemorySpace.PSUM))

        idb = const.tile([128, 128], BF16, tag="idb")
        nc.sync.dma_start(idb[:], ident_b[:])
        ones2 = const.tile([128, 2, 128], FP8, tag="ones2")
        nc.gpsimd.memset(ones2[:], 1.0)

        # ---- memory preprocessing ----
        # Dual-fp8 ldweights needs each [2, 128] stationary block contiguous.
        # memS2[mj][p, ci, i, c] = mem[(2mj+i)*128+p, ci*128+c]   (mm2 lhsT)
        # memT2[cj][p, mt, i, m] = mem_norm[mt*128+m, (2cj+i)*128+p] (mm1 lhsT)
        memS2 = [const.tile([128, C // 128, 2, 128], FP8, tag=f"memS2_{mj}",
                            name=f"memS2_{mj}") for mj in range(MJ)]
        memT2 = [const.tile([128, MC, 2, 128], FP8, tag=f"memT2_{cj}",
                            name=f"memT2_{cj}") for cj in range(CJ)]
        for mi in range(MC):
            mld = mstage.tile([128, C], F32, tag="mld")
            nc.sync.dma_start(mld[:], mem[mi * 128:(mi + 1) * 128, :])
            msq = mtmp.tile([128, C], F32, tag="msq")
            mssq = stats.tile([128, 1], F32, tag="mssq")
            nc.scalar.activation(msq[:], mld[:], AF.Square, accum_out=mssq[:])
            mnorm = stats.tile([128, 1], F32, tag="mnorm")
            nc.scalar.activation(mnorm[:], mssq[:], AF.Sqrt)
            rinv = stats.tile([128, 1], F32, tag="rinv")
            nc.vector.reciprocal(rinv[:], mnorm[:])
            nc.scalar.activation(memS2[mi // 2][:, :, mi % 2, :], mld[:],
                                 AF.Copy)
            mn = mtmp.tile([128, C], BF16, tag="mn")
            nc.vector.tensor_scalar_mul(mn[:], mld[:], rinv[:])
            for ci in range(C // 128):
                ptr = psum.tile([128, 128], BF16, tag="b1", bufs=4,
                                name="ptr")
                nc.tensor.transpose(ptr[:], mn[:, ci * 128:(ci + 1) * 128],
                                    idb[:])
                nc.scalar.activation(
                    memT2[ci // 2][:, mi, ci % 2, :], ptr[:], AF.Copy)

        def phase_dma(b, ns):
            """Prefetch one unit's input (issued a round early)."""
            n0 = ns * 512
            x4 = xio.tile([128, 4, 512], FP8, tag="x4", name="x4")
            for ch in range(4):
                nc.sync.dma_start(x4[:, ch, :],
                                  xs[b, ch * 128:(ch + 1) * 128, n0:n0 + 512])
            return {"b": b, "ns": ns, "x4": x4}

        def phase_a0(st):
            """Squares (scalar, at round start so sumsq can issue early)."""
            xq4 = xio.tile([128, 4, 512], FP8, tag="xq4", name="xq4")
            nc.scalar.activation(xq4[:], st["x4"][:], AF.Square)
            st["xq4"] = xq4

        def phase_a1(st):
            """Sumsq + sim matmuls + copies + column max."""
            x4, xq4 = st["x4"], st["xq4"]
            pq = psum.tile([128, 512], F32, tag="b1", bufs=4, name="pq")
            for cj in range(CJ):
                nc.tensor.matmul(pq[:], ones2[:],
                                 xq4[:, 2 * cj:2 * cj + 2, :],
                                 start=(cj == 0), stop=(cj == CJ - 1),
                                 perf_mode=DR)
            sTb = simb.tile([128, MJ, 2, 512], BF16, tag="sTb", name="sTb")
            for mj in range(MJ):
                ps = psum.tile([128, 2, 512], F32, tag="sim", bufs=2,
                               name="ps")
                for i in range(2):
                    mt = 2 * mj + i
                    for cj in range(CJ):
                        nc.tensor.matmul(
                            ps[:, i, :], memT2[cj][:, mt, :, :],
                            x4[:, 2 * cj:2 * cj + 2, :],
                            start=(cj == 0), stop=(cj == CJ - 1),
                            perf_mode=DR)
                nc.scalar.activation(sTb[:, mj, :, :], ps[:], AF.Copy)
            cmp4 = stats.tile([128, MJ, 512], BF16, tag="cmp4")
            nc.vector.tensor_tensor(cmp4[:], sTb[:, :, 0, :], sTb[:, :, 1, :],
                                    ALU.max)
            cmx2 = stats.tile([128, 2, 512], BF16, tag="cmx2")
            nc.vector.tensor_tensor(cmx2[:], cmp4[:, 0:2, :], cmp4[:, 2:4, :],
                                    ALU.max)
            cm = stats.tile([128, 512], BF16, tag="cm")
            nc.vector.tensor_tensor(cm[:], cmx2[:, 0, :], cmx2[:, 1, :],
                                    ALU.max)
            cmB = stats.tile([128, 512], BF16, tag="cmB", bufs=2)
            nc.gpsimd.partition_all_reduce(cmB[:], cm[:], 128,
                                           bass_isa.ReduceOp.max)
            thr = stats.tile([1, 512], F32, tag="thr")
            nc.scalar.activation(thr[:], pq[0:1, :], AF.Sqrt, scale=THRESH2)
            st["sTb"], st["cmB"], st["thr"] = sTb, cmB, thr

        def phase_c1(st):
            """Fold mask into compare value, broadcast across partitions."""
            cmB, thr = st["cmB"], st["thr"]
            msk = stats.tile([1, 512], F32, tag="msk")
            nc.vector.tensor_tensor(msk[:], cmB[0:1, :], thr[:], ALU.is_le)
            mxrow = stats.tile([1, 512], BF16, tag="mxrow")
            nc.vector.scalar_tensor_tensor(mxrow[:], msk[:], -BIG,
                                           cmB[0:1, :], ALU.mult, ALU.add)
            mxB = stats.tile([128, 512], BF16, tag="mxB", bufs=2)
            nc.gpsimd.partition_broadcast(mxB[:], mxrow[:], 128)
            st["mxB"] = mxB

        def phase_c2(st):
            """Onehot: exact bf16 compare -> fp8 (one mega op)."""
            oh = ohb.tile([128, MJ, 2, 512], FP8, tag="oh", name="oh")
            mxv = st["mxB"][:].unsqueeze(1).unsqueeze(1).broadcast_to(
                [128, MJ, 2, 512])
            nc.vector.tensor_tensor(oh[:], st["sTb"][:], mxv, ALU.is_equal)
            st["oh"] = oh

        def phase_b_mm(st):
            """out[c, n] = sum_m mem[m, c] * onehot[m, n] (psum)."""
            oh = st["oh"]
            st["pB"] = []
            for ci in range(C // 128):
                pB = psum.tile([128, 512], F32, tag="b1", bufs=4, name="pB")
                for mj in range(MJ):
                    nc.tensor.matmul(
                        pB[:], memS2[mj][:, ci, :, :], oh[:, mj, :, :],
                        start=(mj == 0), stop=(mj == MJ - 1), perf_mode=DR)
                st["pB"].append(pB)

        def phase_b_out(st):
            """Drain gather psum -> bf16 sbuf -> DMA (a round later, so
            these never block the scalar queue)."""
            b, ns = st["b"], st["ns"]
            n0 = ns * 512
            for ci in range(C // 128):
                pB = st["pB"][ci]
                ob = ohb.tile([128, 512], BF16, tag="ob", bufs=4, name="ob")
                if ci < 3:
                    nc.scalar.activation(ob[:], pB[:], AF.Copy)
                else:
                    nc.vector.tensor_copy(ob[:], pB[:])
                nc.sync.dma_start(
                    out[b, ci * 128:(ci + 1) * 128, n0:n0 + 512], ob[:])

        # ---- main loop, software-pipelined two units deep ----
        # Round k: dma(k+1), xsq(k), fold+bcast(k-1), gather(k-2),
        #          onehot(k-1), sumsq/sim/max(k).  Each engine's in-order
        #          queue then always sees ready work first.
        units = [(b, ns) for b in range(b_loc) for ns in range(ns_count)]
        states = [None] * len(units)
        states[0] = phase_dma(*units[0])
        for k in range(len(units)):
            if k + 1 < len(units):
                states[k + 1] = phase_dma(*units[k + 1])
            if k >= 3:
                phase_b_out(states[k - 3])
                states[k - 3] = None
            phase_a0(states[k])
            if k >= 1:
                phase_c1(states[k - 1])
                phase_c2(states[k - 1])
            if k >= 2:
                phase_b_mm(states[k - 2])
            phase_a1(states[k])
        last = len(units) - 1
        if last >= 2:
            phase_b_out(states[last - 2])
        phase_c1(states[last])
        phase_c2(states[last])
        if last >= 1:
            phase_b_mm(states[last - 1])
            phase_b_out(states[last - 1])
        phase_b_mm(states[last])
        phase_b_out(states[last])

    nc.compile()
    return nc


_NC_CACHE = {}


def _get_nc(b_loc=B_LOC, n_pix=N_PIX):
    key = (b_loc, n_pix)
    if key not in _NC_CACHE:
        _NC_CACHE[key] = build_kernel(*key)
    return _NC_CACHE[key]


def run_on_hw(x_flat, memory, b_loc=B_LOC, n_pix=N_PIX, trace=False,
              **spmd_kwargs):
    """x_flat: [N_CORES*b_loc, C, n_pix] f32. Returns (out_full, results)."""
    nc = _get_nc(b_loc, n_pix)
    ident_b = np.eye(128, dtype=ml_dtypes.bfloat16)
    x_f8 = x_flat.astype(ml_dtypes.float8_e4m3)
    in_maps = [
        {
            "xs": np.ascontiguousarray(x_f8[c * b_loc:(c + 1) * b_loc]),
            "memory": memory,
            "identity": ident_b,
        }
        for c in range(N_CORES)
    ]
    res = run_bass_kernel_spmd(nc, in_maps, list(range(N_CORES)),
                               trace=trace, **spmd_kwargs)
    outs = [np.asarray(res.results[c]["out"]).astype(np.float32)
            for c in range(N_CORES)]
    return np.concatenate(outs, axis=0), res


def kernel(x, memory):
    x = np.asarray(x, dtype=np.float32)
    memory = np.asarray(memory, dtype=np.float32)
    B, C_, H_, W_ = x.shape
    x_flat = np.ascontiguousarray(x.reshape(B, C_, H_ * W_))
    out_flat, _ = run_on_hw(x_flat, memory)
    return out_flat.reshape(B, C_, H_, W_)


# revision 27
# speedup vs baseline: 1.0752x; 1.0091x over previous
"""HardMemory retrieval-KNN kernel for 8 Trainium2 NeuronCores.

Data-parallel: 32 batches sharded 4-per-core; memory bank [1024,512]
replicated. Per batch b (x_b = [C=512, N=4096]), processed in eight
512-pixel units, software-pipelined two units deep so no engine waits
on the cross-engine compare chain:

  round k emits:  A(k)  = DMA + squares + sumsq/sim fp8 DoubleRow
                          matmuls + psum->sbuf bf16 copies + DVE max
                          tree + gpsimd partition max
                  C1(k-1) = threshold fold + gpsimd broadcast
                  B(k-2)  = gather matmul (fp8 DR) + out copies + DMA
                  C2(k-1) = onehot compare (bf16 exact -> fp8)

  simT[m,n]  = <x_n, mem_m/||mem_m||>    fp8 DR matmul, f32 psum
  thr[n]     = 0.8*sqrt(sum_c x^2)       ones-stationary fp8 DR matmul
  cm[n]      = colmax_m bf16(simT)       DVE bf16 2x + gpsimd reduce
  mx'[n]     = cm - BIG*(cm <= thr)      mask folded into compare value
  oh[m,n]    = (bf16(simT) == bcast(mx'))
  out[:,n]   = memory^T @ oh             fp8 DR matmul -> bf16 out

x arrives as fp8e4m3 (host cast): halves input DMA and enables the
DoubleRow similarity matmul.  Cosine margins are huge vs fp8 noise
(|sim| <= ~6 vs thr ~18 for randn inputs), and the bf16 compare domain
is exact by construction (max of bf16 values == some bf16 value).
"""

import sys

for _p in ("/opt/trn_rl_repo",):
    if _p not in sys.path:
        sys.path.insert(0, _p)

from contextlib import ExitStack

import ml_dtypes
import numpy as np

import concourse.bass as bass
import concourse.tile as tile
from concourse import bacc, bass_isa, mybir
from concourse.bass_utils import run_bass_kernel_spmd

F32 = mybir.dt.float32
BF16 = mybir.dt.bfloat16
FP8 = mybir.dt.float8e4
AF = mybir.ActivationFunctionType
ALU = mybir.AluOpType
DR = mybir.MatmulPerfMode.DoubleRow

B_FULL, C, H, W = 32, 512, 64, 64
N_PIX = H * W
M = 1024
N_CORES = 8
B_LOC = B_FULL // N_CORES
THRESH2 = 0.8 * 0.8
BIG = 1.0e30

MC = M // 128            # 8 memory chunks
MJ = MC // 2             # 4 DoubleRow memory pairs
CJ = C // 256            # 2 DoubleRow contraction pairs


def build_kernel(b_loc=B_LOC, n_pix=N_PIX):
    ns_count = n_pix // 512

    nc = bacc.Bacc("TRN2", target_bir_lowering=False, debug=False,
                   num_devices=N_CORES)
    xs = nc.dram_tensor("xs", [b_loc, C, n_pix], FP8, kind="ExternalInput")
    mem = nc.dram_tensor("memory", [M, C], F32, kind="ExternalInput")
    ident_b = nc.dram_tensor("identity", [128, 128], BF16, kind="ExternalInput")
    out = nc.dram_tensor("out", [b_loc, C, n_pix], BF16,
                         kind="ExternalOutput")

    with tile.TileContext(nc) as tc, ExitStack() as ctx:
        const = ctx.enter_context(tc.tile_pool(name="const", bufs=1))
        mstage = ctx.enter_context(tc.tile_pool(name="mstage", bufs=2))
        mtmp = ctx.enter_context(tc.tile_pool(name="mtmp", bufs=2))
        xio = ctx.enter_context(tc.tile_pool(name="xio", bufs=6))
        simb = ctx.enter_context(tc.tile_pool(name="simb", bufs=4))
        ohb = ctx.enter_context(tc.tile_pool(name="ohb", bufs=4))
        stats = ctx.enter_context(tc.tile_pool(name="stats", bufs=6))
        # psum (8 banks): sim 2x[128,2,512]f32 (4) + b1 4x[128,512]f32 (4);
        # preproc transposes ride the b1 ring.
        psum = ctx.enter_context(
            tc.tile_pool(name="psum", bufs=1, space=bass.MemorySpace.PSUM))

        idb = const.tile([128, 128], BF16, tag="idb")
        nc.sync.dma_start(idb[:], ident_b[:])
        ones2 = const.tile([128, 2, 128], FP8, tag="ones2")
        nc.gpsimd.memset(ones2[:], 1.0)

        # ---- memory preprocessing ----
        # Dual-fp8 ldweights needs each [2, 128] stationary block contiguous.
        # memS2[mj][p, ci, i, c] = mem[(2mj+i)*128+p, ci*128+c]   (mm2 lhsT)
        # memT2[cj][p, mt, i, m] = mem_norm[mt*128+m, (2cj+i)*128+p] (mm1 lhsT)
        memS2 = [const.tile([128, C // 128, 2, 128], FP8, tag=f"memS2_{mj}",
                            name=f"memS2_{mj}") for mj in range(MJ)]
        memT2 = [const.tile([128, MC, 2, 128], FP8, tag=f"memT2_{cj}",
                            name=f"memT2_{cj}") for cj in range(CJ)]
        for mi in range(MC):
            mld = mstage.tile([128, C], F32, tag="mld")
            nc.sync.dma_start(mld[:], mem[mi * 128:(mi + 1) * 128, :])
            msq = mtmp.tile([128, C], F32, tag="msq")
            mssq = stats.tile([128, 1], F32, tag="mssq")
            nc.scalar.activation(msq[:], mld[:], AF.Square, accum_out=mssq[:])
            mnorm = stats.tile([128, 1], F32, tag="mnorm")
            nc.scalar.activation(mnorm[:], mssq[:], AF.Sqrt)
            rinv = stats.tile([128, 1], F32, tag="rinv")
            nc.vector.reciprocal(rinv[:], mnorm[:])
            nc.scalar.activation(memS2[mi // 2][:, :, mi % 2, :], mld[:],
                                 AF.Copy)
            mn = mtmp.tile([128, C], BF16, tag="mn")
            nc.vector.tensor_scalar_mul(mn[:], mld[:], rinv[:])
            for ci in range(C // 128):
                ptr = psum.tile([128, 128], BF16, tag="b1", bufs=4,
                                name="ptr")
                nc.tensor.transpose(ptr[:], mn[:, ci * 128:(ci + 1) * 128],
                                    idb[:])
                nc.scalar.activation(
                    memT2[ci // 2][:, mi, ci % 2, :], ptr[:], AF.Copy)

        def phase_dma(b, ns):
            """Prefetch one unit's input (issued a round early)."""
            n0 = ns * 512
            x4 = xio.tile([128, 4, 512], FP8, tag="x4", name="x4")
            for ch in range(4):
                nc.sync.dma_start(x4[:, ch, :],
                                  xs[b, ch * 128:(ch + 1) * 128, n0:n0 + 512])
            return {"b": b, "ns": ns, "x4": x4}

        def phase_a0(st):
            """Squares (scalar, at round start so sumsq can issue early)."""
            xq4 = xio.tile([128, 4, 512], FP8, tag="xq4", name="xq4")
            nc.scalar.activation(xq4[:], st["x4"][:], AF.Square)
            st["xq4"] = xq4

        def phase_a1(st):
            """Sumsq + sim matmuls + copies + column max."""
            x4, xq4 = st["x4"], st["xq4"]
            pq = psum.tile([128, 512], F32, tag="b1", bufs=4, name="pq")
            for cj in range(CJ):
                nc.tensor.matmul(pq[:], ones2[:],
                                 xq4[:, 2 * cj:2 * cj + 2, :],
                                 start=(cj == 0), stop=(cj == CJ - 1),
                                 perf_mode=DR)
            sTb = simb.tile([128, MJ, 2, 512], BF16, tag="sTb", name="sTb")
            for mj in range(MJ):
                ps = psum.tile([128, 2, 512], F32, tag="sim", bufs=2,
                               name="ps")
                for i in range(2):
                    mt = 2 * mj + i
                    for cj in range(CJ):
                        nc.tensor.matmul(
                            ps[:, i, :], memT2[cj][:, mt, :, :],
                            x4[:, 2 * cj:2 * cj + 2, :],
                            start=(cj == 0), stop=(cj == CJ - 1),
                            perf_mode=DR)
                nc.scalar.activation(sTb[:, mj, :, :], ps[:], AF.Copy)
            cmp4 = stats.tile([128, MJ, 512], BF16, tag="cmp4")
            nc.vector.tensor_tensor(cmp4[:], sTb[:, :, 0, :], sTb[:, :, 1, :],
                                    ALU.max)
            cmx2 = stats.tile([128, 2, 512], BF16, tag="cmx2")
            nc.vector.tensor_tensor(cmx2[:], cmp4[:, 0:2, :], cmp4[:, 2:4, :],
                                    ALU.max)
            cm = stats.tile([128, 512], BF16, tag="cm")
            nc.vector.tensor_tensor(cm[:], cmx2[:, 0, :], cmx2[:, 1, :],
                                    ALU.max)
            cmB = stats.tile([128, 512], BF16, tag="cmB", bufs=3)
            nc.gpsimd.partition_all_reduce(cmB[:], cm[:], 128,
                                           bass_isa.ReduceOp.max)
            thr = stats.tile([1, 512], F32, tag="thr")
            nc.scalar.activation(thr[:], pq[0:1, :], AF.Sqrt, scale=THRESH2)
            st["sTb"], st["cmB"], st["thr"] = sTb, cmB, thr

        def phase_c1(st):
            """Fold mask into compare value, broadcast across partitions."""
            cmB, thr = st["cmB"], st["thr"]
            msk = stats.tile([1, 512], F32, tag="msk")
            nc.vector.tensor_tensor(msk[:], cmB[0:1, :], thr[:], ALU.is_le)
            mxrow = stats.tile([1, 512], BF16, tag="mxrow")
            nc.vector.scalar_tensor_tensor(mxrow[:], msk[:], -BIG,
                                           cmB[0:1, :], ALU.mult, ALU.add)
            mxB = stats.tile([128, 512], BF16, tag="mxB", bufs=3)
            nc.gpsimd.partition_broadcast(mxB[:], mxrow[:], 128)
            st["mxB"] = mxB

        def phase_c2(st):
            """Onehot: exact bf16 compare -> fp8 (one mega op)."""
            oh = ohb.tile([128, MJ, 2, 512], FP8, tag="oh", name="oh")
            mxv = st["mxB"][:].unsqueeze(1).unsqueeze(1).broadcast_to(
                [128, MJ, 2, 512])
            nc.vector.tensor_tensor(oh[:], st["sTb"][:], mxv, ALU.is_equal)
            st["oh"] = oh

        def phase_b_mm(st):
            """out[c, n] = sum_m mem[m, c] * onehot[m, n] (psum)."""
            oh = st["oh"]
            st["pB"] = []
            for ci in range(C // 128):
                pB = psum.tile([128, 512], F32, tag="b1", bufs=4, name="pB")
                for mj in range(MJ):
                    nc.tensor.matmul(
                        pB[:], memS2[mj][:, ci, :, :], oh[:, mj, :, :],
                        start=(mj == 0), stop=(mj == MJ - 1), perf_mode=DR)
                st["pB"].append(pB)

        def phase_b_out(st):
            """Drain gather psum -> bf16 sbuf -> DMA (a round later, so
            these never block the scalar queue)."""
            b, ns = st["b"], st["ns"]
            n0 = ns * 512
            for ci in range(C // 128):
                pB = st["pB"][ci]
                ob = ohb.tile([128, 512], BF16, tag="ob", bufs=8, name="ob")
                if ci < 3:
                    nc.scalar.activation(ob[:], pB[:], AF.Copy)
                else:
                    nc.vector.tensor_copy(ob[:], pB[:])
                nc.sync.dma_start(
                    out[b, ci * 128:(ci + 1) * 128, n0:n0 + 512], ob[:])

        # ---- main loop, software-pipelined two units deep ----
        # Round k: dma(k+1), xsq(k), fold+bcast(k-1), gather(k-2),
        #          onehot(k-1), sumsq/sim/max(k).  Each engine's in-order
        #          queue then always sees ready work first.
        units = [(b, ns) for b in range(b_loc) for ns in range(ns_count)]
        states = [None] * len(units)
        states[0] = phase_dma(*units[0])
        for k in range(len(units)):
            if k + 1 < len(units):
                states[k + 1] = phase_dma(*units[k + 1])
            if k >= 3:
                phase_b_out(states[k - 3])
                states[k - 3] = None
            phase_a0(states[k])
            if k >= 1:
                phase_c1(states[k - 1])
                phase_c2(states[k - 1])
            if k >= 2:
                phase_b_mm(states[k - 2])
            phase_a1(states[k])
        last = len(units) - 1
        if last >= 2:
            phase_b_out(states[last - 2])
        phase_c1(states[last])
        phase_c2(states[last])
        if last >= 1:
            phase_b_mm(states[last - 1])
            phase_b_out(states[last - 1])
        phase_b_mm(states[last])
        phase_b_out(states[last])

    nc.compile()
    return nc


_NC_CACHE = {}


def _get_nc(b_loc=B_LOC, n_pix=N_PIX):
    key = (b_loc, n_pix)
    if key not in _NC_CACHE:
        _NC_CACHE[key] = build_kernel(*key)
    return _NC_CACHE[key]


def run_on_hw(x_flat, memory, b_loc=B_LOC, n_pix=N_PIX, trace=False,
              **spmd_kwargs):
    """x_flat: [N_CORES*b_loc, C, n_pix] f32. Returns (out_full, results)."""
    nc = _get_nc(b_loc, n_pix)
    ident_b = np.eye(128, dtype=ml_dtypes.bfloat16)
    x_f8 = x_flat.astype(ml_dtypes.float8_e4m3)
    in_maps = [
        {
            "xs": np.ascontiguousarray(x_f8[c * b_loc:(c + 1) * b_loc]),
            "memory": memory,
            "identity": ident_b,
        }
        for c in range(N_CORES)
    ]
    res = run_bass_kernel_spmd(nc, in_maps, list(range(N_CORES)),
                               trace=trace, **spmd_kwargs)
    outs = [np.asarray(res.results[c]["out"]).astype(np.float32)
            for c in range(N_CORES)]
    return np.concatenate(outs, axis=0), res


def kernel(x, memory):
    x = np.asarray(x, dtype=np.float32)
    memory = np.asarray(memory, dtype=np.float32)
    B, C_, H_, W_ = x.shape
    x_flat = np.ascontiguousarray(x.reshape(B, C_, H_ * W_))
    out_flat, _ = run_on_hw(x_flat, memory)
    return out_flat.reshape(B, C_, H_, W_)
